# revision 13
# baseline (speedup 1.0000x reference)
"""ContrastiveLoss (nn_ContrastiveLoss_17093969838495) Trainium2 kernel.

Math: for p1, p2 in R^{BxD} the reference computes
    pos_loss = sum((p1-p2)^2)/B
    d[i,j]   = ||p1_i||^2 + ||p2_j||^2 - 2 <p1_i, p2_j>
    neg_loss = -(sum(d) - trace(d)) / (B*(B-1))
    out      = pos_loss + neg_loss

The BxB matrix is never needed:
    sum(d)   = B*sum(p1^2) + B*sum(p2^2) - 2 * (colsum(p1) . colsum(p2))
    trace(d) = sum(p1^2) + sum(p2^2) - 2*sum(p1 * p2) = sum((p1-p2)^2)

So each core only reduces its 512-row block: sums of squares, sum of
products, and per-column sums. The kernel is input-DMA bound, so inputs
are cast to fp16 on the host before transfer (quantizing inputs AND
products to fp16 moves the final loss by ~3e-6 relative; the gate is
2e-2). All on-device accumulation stays fp32.

Engine split (fp16 keeps DVE in its fast 2x tensor_tensor mode;
scalar_tensor_tensor is 1x-only, so it is used only on tiny tail spans):
  ACT : sum(p1^2) via fused Square+accumulate (one pass, no forms)
  DVE : forms p1*p2 (and the p2^2 tiles POOL cannot absorb) with 2x TT
  POOL: forms p2^2 for the first two row-tiles (otherwise idle)
  PE  : reduces every formed tile AND the raw colsums with
        data-stationary ones-vector matmuls into separate PSUM columns
        (PSUM matmul accumulation flags corrupt data on this toolchain,
        so every matmul gets its own column and the HOST does the final
        fold); DVE/ACT batch-copy raw PSUM columns into the output tile.
Row-tiles are DMA'd in column spans; the final tile's spans shrink so
the compute tail past the last byte stays short. Host combines the 8
per-core [128, OUT_COLS] partials in float64.
"""

import numpy as np

try:
    import concourse.bass as bass
except ImportError:  # pragma: no cover - path fallback for fresh dirs
    import sys

    sys.path.insert(0, "/opt/trn_rl_repo")
    import concourse.bass as bass

import concourse.bacc as bacc
import concourse.tile as tile
from concourse import mybir
from concourse.bass_utils import run_bass_kernel_spmd

N_CORES = 8
B = 4096
D = 4096
RB = B // N_CORES  # 512 rows per core
P = 128  # SBUF partitions
NT = RB // P  # 4 row-tiles per core
NCH = D // P  # 32 column chunks of 128

# ---- output layout ----
# [0:NDIR)                direct accum columns (ACT fused / DVE STT)
#   sq1: [0:12)  sq2: [12:18)  prd: [18:24)
# [NDIR : NDIR+NRED)      PE-reduced chunk columns of formed tiles
#   prd chunks: [NDIR : NDIR+120)   sq2 chunks: [NDIR+120 : NDIR+240)
# [CS0 : CS0+256)         raw per-tile colsums, cs[t] at CS0+64*t;
#                         within: p1 chunk j col j, p2 chunk j col 32+j
NDIR = 24
DIR_SQ1, DIR_SQ2, DIR_PRD = 0, 12, 18
NRED = 240
RED0 = NDIR
RED_PRD = RED0  # 120 cols
RED_SQ2 = RED0 + 120  # 120 cols
CS0 = NDIR + NRED  # 264
OUT_COLS = CS0 + NT * 2 * NCH  # 264 + 256 = 520

_CACHE = {}


def build_program(replicas=1):
    f16 = mybir.dt.float16
    f32 = mybir.dt.float32
    nc = bacc.Bacc(
        "TRN2", target_bir_lowering=False, debug=False, num_devices=N_CORES
    )
    p1 = nc.dram_tensor("p1", [RB, D], f16, kind="ExternalInput")
    p2 = nc.dram_tensor("p2", [RB, D], f16, kind="ExternalInput")
    out = nc.dram_tensor("out", [P, OUT_COLS], f32, kind="ExternalOutput")

    with tile.TileContext(nc) as tc:
        with (
            tc.tile_pool(name="in1", bufs=4) as pool1,
            tc.tile_pool(name="in2", bufs=4) as pool2,
            tc.tile_pool(name="ascr", bufs=2) as ascr,
            tc.tile_pool(name="dscr", bufs=3) as dscr,
            tc.tile_pool(name="pscr", bufs=3) as pscr,
            tc.tile_pool(name="misc", bufs=1) as misc,
            tc.tile_pool(name="outp", bufs=2) as outp,
            tc.tile_pool(name="psum", bufs=1, space=bass.MemorySpace.PSUM) as psp,
        ):
            scr = {"act": ascr, "dve": dscr, "pool": pscr}
            ones = misc.tile([P, 1], f16)
            for _rep in range(replicas):
                _build_body(nc, pool1, pool2, scr, misc, outp, psp, ones, p1, p2, out)

    nc.compile()
    return nc


def _build_body(nc, pool1, pool2, scr, misc, outp, psp, ones, p1, p2, out):
    f16 = mybir.dt.float16
    f32 = mybir.dt.float32
    out_sb = outp.tile([P, OUT_COLS], f32, tag="out_sb")
    # one 496-col PSUM strip: PE-reduce chunk cols [0:240), colsums [240:496)
    ps = psp.tile([P, 496], f32, tag="ps")
    PS_RED = 0
    PS_CS = 240

    dir_n = {"sq1": DIR_SQ1, "sq2": DIR_SQ2, "prd": DIR_PRD}
    red_n = {"sq2": RED_SQ2 - RED0, "prd": RED_PRD - RED0}

    def acc_col(q):
        c = dir_n[q]
        dir_n[q] += 1
        return out_sb[:, c : c + 1]

    def act_square(src, q):
        s = scr["act"].tile([P, src.shape[-1]], f16, tag="ascr")
        nc.scalar.activation(
            s[:],
            src,
            mybir.ActivationFunctionType.Square,
            accum_out=acc_col(q),
        )

    def dve_stt(a, b, q):
        # single-instruction multiply+accumulate (1x mode; tiny spans only)
        s = scr["dve"].tile([P, a.shape[-1]], f16, tag="dscr")
        nc.vector.scalar_tensor_tensor(
            out=s[:],
            in0=a,
            scalar=1.0,
            in1=b,
            op0=mybir.AluOpType.mult,
            op1=mybir.AluOpType.mult,
            accum_out=acc_col(q),
        )

    def pe_reduce(s, q):
        """Column-sum the formed tile into fresh PSUM chunk columns."""
        cw = s.shape[-1]
        nch = cw // P
        base = PS_RED + red_n[q]
        red_n[q] += nch
        assert red_n[q] <= (RED_SQ2 - RED0 if q == "prd" else NRED), q
        for j in range(nch):
            nc.tensor.matmul(
                ps[:, base + j : base + j + 1], s[:, j * P : (j + 1) * P], ones[:]
            )

    def form(engine, a, b, q):
        pool = scr["dve"] if engine == "dve" else scr["pool"]
        s = pool.tile([P, a.shape[-1]], f16, tag=engine + "scr")
        if engine == "dve":
            nc.vector.tensor_tensor(out=s[:], in0=a, in1=b, op=mybir.AluOpType.mult)
        else:
            nc.gpsimd.tensor_tensor(out=s[:], in0=a, in1=b, op=mybir.AluOpType.mult)
        pe_reduce(s, q)

    def colsums(t, p1t, p2t, jlo, jhi):
        base = PS_CS + t * 2 * NCH
        for j in range(jlo, jhi):
            nc.tensor.matmul(
                ps[:, base + j : base + j + 1],
                p1t[:, j * P : (j + 1) * P],
                ones[:],
            )
            nc.tensor.matmul(
                ps[:, base + NCH + j : base + NCH + j + 1],
                p2t[:, j * P : (j + 1) * P],
                ones[:],
            )

    def dve_copy(ps_lo, ps_hi, sb_lo):
        nc.vector.tensor_copy(
            out_sb[:, sb_lo : sb_lo + (ps_hi - ps_lo)], ps[:, ps_lo:ps_hi]
        )

    tiles = []
    for t in range(NT):
        p1t = pool1.tile([P, D], f16, tag="p1t")
        p2t = pool2.tile([P, D], f16, tag="p2t")
        tiles.append((p1t, p2t))

    def dma_span(t, sl):
        rows = slice(t * P, (t + 1) * P)
        nc.sync.dma_start(out=tiles[t][0][:, sl], in_=p1[rows, sl])
        nc.sync.dma_start(out=tiles[t][1][:, sl], in_=p2[rows, sl])

    H1, H2 = slice(0, 2048), slice(2048, 4096)

    # --- tile 0 DMAs first so transfers start before any setup ops ---
    dma_span(0, H1)
    dma_span(0, H2)
    nc.vector.memset(ones[:], 1.0)
    # unused direct-accum columns must read as zero in the host combine
    nc.gpsimd.memset(out_sb[:, 0:NDIR], 0.0)
    # preload the ACT table during the DMA head (dummy square on ones)
    warm = misc.tile([P, 1], f16)
    nc.scalar.activation(warm[:], ones[:], mybir.ActivationFunctionType.Square)

    p1t, p2t = tiles[0]
    act_square(p1t[:, H1], "sq1")
    form("dve", p1t[:, H1], p2t[:, H1], "prd")  # prd t0s0
    form("pool", p2t[:, H1], p2t[:, H1], "sq2")  # sq2 t0s0 (POOL)
    colsums(0, p1t, p2t, 0, 16)
    act_square(p1t[:, H2], "sq1")
    form("dve", p1t[:, H2], p2t[:, H2], "prd")  # prd t0s1
    form("pool", p2t[:, H2], p2t[:, H2], "sq2")  # sq2 t0s1 (POOL)
    colsums(0, p1t, p2t, 16, 32)

    # --- tile 1 ---
    dma_span(1, H1)
    dma_span(1, H2)
    p1t, p2t = tiles[1]
    act_square(p1t[:, 0:4096], "sq1")
    form("dve", p1t[:, 0:4096], p2t[:, 0:4096], "prd")  # prd t1
    form("pool", p2t[:, H1], p2t[:, H1], "sq2")  # sq2 t1s0 (POOL)
    form("pool", p2t[:, H2], p2t[:, H2], "sq2")  # sq2 t1s1 (POOL)
    colsums(1, p1t, p2t, 0, 32)
    # batch copy 1: everything formed from tile 0 (prd 0:32, sq2 0:32)
    dve_copy(PS_RED + 0, PS_RED + 32, RED_PRD)
    dve_copy(PS_RED + 120, PS_RED + 152, RED_SQ2)
    dve_copy(PS_CS + 0, PS_CS + 64, CS0)  # cs t0

    # --- tile 2 ---
    dma_span(2, H1)
    dma_span(2, H2)
    p1t, p2t = tiles[2]
    act_square(p1t[:, 0:4096], "sq1")
    form("dve", p1t[:, 0:4096], p2t[:, 0:4096], "prd")  # prd t2
    form("dve", p2t[:, H1], p2t[:, H1], "sq2")  # sq2 t2s0 (DVE)
    form("dve", p2t[:, H2], p2t[:, H2], "sq2")  # sq2 t2s1 (DVE)
    colsums(2, p1t, p2t, 0, 32)
    # batch copy 2: tile-1 products + POOL tile-1 squares + cs t1
    dve_copy(PS_RED + 32, PS_RED + 64, RED_PRD + 32)
    dve_copy(PS_RED + 152, PS_RED + 184, RED_SQ2 + 32)
    dve_copy(PS_CS + 64, PS_CS + 128, CS0 + 64)  # cs t1

    # --- tile 3 (tail): small spans, tiny direct-accum ops at the end ---
    t = 3
    p1t, p2t = tiles[t]
    sls = []
    off = 0
    for cw in (1024, 1024, 1024, 512, 256, 256):
        sl = slice(off, off + cw)
        off += cw
        sls.append(sl)
        dma_span(t, sl)
    s0, s1, s2, s3, s4, s5 = sls

    act_square(p1t[:, 0:2048], "sq1")  # covers s0+s1
    form("dve", p1t[:, 0:2048], p2t[:, 0:2048], "prd")
    form("dve", p2t[:, 0:2048], p2t[:, 0:2048], "sq2")
    colsums(3, p1t, p2t, 0, 16)
    act_square(p1t[:, s2], "sq1")
    form("dve", p1t[:, s2], p2t[:, s2], "prd")
    form("dve", p2t[:, s2], p2t[:, s2], "sq2")
    colsums(3, p1t, p2t, 16, 24)
    # batch copy 3: tile-2 products + DVE tile-2 squares + cs t2
    dve_copy(PS_RED + 64, PS_RED + 96, RED_PRD + 64)
    dve_copy(PS_RED + 184, PS_RED + 216, RED_SQ2 + 32 + 32)
    dve_copy(PS_CS + 128, PS_CS + 192, CS0 + 128)  # cs t2
    # tail spans: direct accumulate, smallest last
    act_square(p1t[:, s3], "sq1")
    dve_stt(p1t[:, s3], p2t[:, s3], "prd")
    dve_stt(p2t[:, s3], p2t[:, s3], "sq2")
    colsums(3, p1t, p2t, 24, 28)
    act_square(p1t[:, s4], "sq1")
    dve_stt(p1t[:, s4], p2t[:, s4], "prd")
    act_square(p2t[:, s4], "sq2")
    colsums(3, p1t, p2t, 28, 30)
    act_square(p1t[:, s5], "sq1")
    dve_stt(p1t[:, s5], p2t[:, s5], "prd")
    dve_stt(p2t[:, s5], p2t[:, s5], "sq2")
    colsums(3, p1t, p2t, 30, 32)

    # batch copy 4: tile-3 formed chunks + cs t3
    dve_copy(PS_RED + 96, PS_RED + 120, RED_PRD + 96)
    dve_copy(PS_RED + 216, PS_RED + 240, RED_SQ2 + 96)
    dve_copy(PS_CS + 192, PS_CS + 256, CS0 + 192)  # cs t3

    nc.sync.dma_start(out=out[:, :], in_=out_sb[:])


def _get_program():
    if "nc" not in _CACHE:
        _CACHE["nc"] = build_program()
    return _CACHE["nc"]


def run_device(p1, p2, trace=False):
    """Run the SPMD kernel; returns (per-core outs list, BassKernelResults)."""
    nc = _get_program()
    h1 = p1.astype(np.float16)
    h2 = p2.astype(np.float16)
    in_maps = [
        {
            "p1": np.ascontiguousarray(h1[c * RB : (c + 1) * RB]),
            "p2": np.ascontiguousarray(h2[c * RB : (c + 1) * RB]),
        }
        for c in range(N_CORES)
    ]
    try:
        bres = run_bass_kernel_spmd(nc, in_maps, list(range(N_CORES)), trace=trace)
    except ModuleNotFoundError:
        # axon NTFF profile hook unavailable in this image; run untraced
        import os

        os.environ["BASS_NEVER_TRACE"] = "1"
        bres = run_bass_kernel_spmd(nc, in_maps, list(range(N_CORES)), trace=False)
    except Exception:
        # transient device wedge (NRT_EXEC_UNIT_UNRECOVERABLE) recovers after
        # a short wait; retry once before giving up
        import time

        time.sleep(30)
        bres = run_bass_kernel_spmd(nc, in_maps, list(range(N_CORES)), trace=False)
    return [r["out"] for r in bres.results], bres


def combine_partials(outs):
    """float64 combine of the per-core [P, OUT_COLS] partials -> f32 scalar."""
    total = np.zeros((P, OUT_COLS), np.float64)
    for o in outs:
        total += o.astype(np.float64)

    # scalar sums: direct accum columns + all PE-reduced chunk columns
    n1 = total[:, DIR_SQ1:DIR_SQ2].sum()
    n2 = total[:, DIR_SQ2:DIR_PRD].sum() + total[:, RED_SQ2 : RED_SQ2 + 120].sum()
    pp = total[:, DIR_PRD:NDIR].sum() + total[:, RED_PRD : RED_PRD + 120].sum()

    # colsums: cs[t] at CS0+64t, p1 chunk j in col j, p2 in col 32+j;
    # entry [m, j] is colsum of column j*128+m
    cs = total[:, CS0:OUT_COLS].reshape(P, NT, 2 * NCH).sum(axis=1)  # [128, 64]
    s1 = cs[:, 0:NCH].T.reshape(-1)
    s2 = cs[:, NCH : 2 * NCH].T.reshape(-1)

    S = n1 + n2 - 2.0 * pp  # sum((p1-p2)^2) == trace(d)
    d_sum = B * (n1 + n2) - 2.0 * (s1 @ s2)
    off = d_sum - S
    result = S / B - off / (B * (B - 1))
    return np.asarray(result, dtype=np.float32)


def kernel(postive1, postive2):
    p1 = np.ascontiguousarray(np.asarray(postive1, dtype=np.float32))
    p2 = np.ascontiguousarray(np.asarray(postive2, dtype=np.float32))
    assert p1.shape == (B, D) and p2.shape == (B, D)
    outs, _ = run_device(p1, p2, trace=False)
    return combine_partials(outs)


# revision 18
# speedup vs baseline: 1.0476x; 1.0476x over previous
"""ContrastiveLoss (nn_ContrastiveLoss_17093969838495) Trainium2 kernel.

Math: for p1, p2 in R^{BxD} the reference computes
    pos_loss = sum((p1-p2)^2)/B
    d[i,j]   = ||p1_i||^2 + ||p2_j||^2 - 2 <p1_i, p2_j>
    neg_loss = -(sum(d) - trace(d)) / (B*(B-1))
    out      = pos_loss + neg_loss

The BxB matrix is never needed:
    sum(d)   = B*sum(p1^2) + B*sum(p2^2) - 2 * (colsum(p1) . colsum(p2))
    trace(d) = sum(p1^2) + sum(p2^2) - 2*sum(p1 * p2) = sum((p1-p2)^2)

So each core only reduces its 512-row block: sums of squares, sum of
products, and per-column sums. The kernel is input-DMA bound, so inputs
are cast to fp16 on the host before transfer (quantizing inputs AND
products to fp16 moves the final loss by ~3e-6 relative; the gate is
2e-2). All on-device accumulation stays fp32.

Engine split (fp16 keeps DVE in its fast 2x tensor_tensor mode;
scalar_tensor_tensor is 1x-only, so it is used only on tiny tail spans):
  ACT : sum(p1^2) via fused Square+accumulate (one pass, no forms)
  DVE : forms p1*p2 (and the p2^2 spans POOL cannot absorb) with 2x TT
  POOL: forms p2^2 for the first two row-tiles (otherwise idle)
  PE  : reduces every formed tile AND the raw colsums with
        data-stationary ones-vector matmuls into separate PSUM columns
        (PSUM matmul accumulation flags corrupt data on this toolchain,
        so every matmul gets its own column and the HOST does the final
        fold).
PSUM is bank-granular, so each era (row-tile) owns one PSUM bank laid
out [products | squares | colsums]; DVE copies each era's bank to the
output tile once, right after the era's last matmul. All compute is
per-span so each op starts as soon as its bytes land; the final tile's
spans taper so the tail past the last byte stays short. Tail-written
output columns sit contiguously in [0:122) and leave in a tiny late
DMA; the bulk leaves early. Host combines the 8 per-core
[128, OUT_COLS] partials in float64.
"""

import numpy as np

try:
    import concourse.bass as bass
except ImportError:  # pragma: no cover - path fallback for fresh dirs
    import sys

    sys.path.insert(0, "/opt/trn_rl_repo")
    import concourse.bass as bass

import concourse.bacc as bacc
import concourse.tile as tile
from concourse import mybir
from concourse.bass_utils import run_bass_kernel_spmd

N_CORES = 8
B = 4096
D = 4096
RB = B // N_CORES  # 512 rows per core
P = 128  # SBUF partitions
NT = RB // P  # 4 row-tiles per core
NCH = D // P  # 32 column chunks of 128

# ---- output layout (tail-written columns first, bulk after) ----
# direct accum columns
DIR_SQ1 = 0  # 12 cols
DIR_SQ2 = 12  # 6 cols
DIR_PRD = 18  # 6 cols (4 used)
# era-3 bank image: pr [24:44), sq [44:58) ([44:52)=p2^2 s0, [52:58)=p1^2 s1),
# colsums [58:122)
E3 = 24
TAIL_END = 122
# era 0-2 bank images: pr [+0:32), sq [+32:64), colsums [+64:128)
EB = (128, 256, 384)
OUT_COLS = 512

_CACHE = {}


def build_program(replicas=1):
    f16 = mybir.dt.float16
    f32 = mybir.dt.float32
    nc = bacc.Bacc(
        "TRN2", target_bir_lowering=False, debug=False, num_devices=N_CORES
    )
    p1 = nc.dram_tensor("p1", [RB, D], f16, kind="ExternalInput")
    p2 = nc.dram_tensor("p2", [RB, D], f16, kind="ExternalInput")
    out = nc.dram_tensor("out", [P, OUT_COLS], f32, kind="ExternalOutput")

    with tile.TileContext(nc) as tc:
        with (
            tc.tile_pool(name="in1", bufs=4) as pool1,
            tc.tile_pool(name="in2", bufs=4) as pool2,
            tc.tile_pool(name="ascr", bufs=2) as ascr,
            tc.tile_pool(name="dscr", bufs=3) as dscr,
            tc.tile_pool(name="pscr", bufs=3) as pscr,
            tc.tile_pool(name="misc", bufs=1) as misc,
            tc.tile_pool(name="outp", bufs=2) as outp,
            tc.tile_pool(name="psum", bufs=1, space=bass.MemorySpace.PSUM) as psp,
        ):
            scr = {"act": ascr, "dve": dscr, "pool": pscr}
            ones = misc.tile([P, 1], f16)
            for _rep in range(replicas):
                _build_body(nc, pool1, pool2, scr, misc, outp, psp, ones, p1, p2, out)

    nc.compile()
    return nc


def _build_body(nc, pool1, pool2, scr, misc, outp, psp, ones, p1, p2, out):
    f16 = mybir.dt.float16
    f32 = mybir.dt.float32
    out_sb = outp.tile([P, OUT_COLS], f32, tag="out_sb")

    dir_n = {"sq1": DIR_SQ1, "sq2": DIR_SQ2, "prd": DIR_PRD}

    def acc_col(q):
        c = dir_n[q]
        dir_n[q] += 1
        assert c < (12, 18, 24)[("sq1", "sq2", "prd").index(q)], q
        return out_sb[:, c : c + 1]

    def act_square(src, q):
        s = scr["act"].tile([P, src.shape[-1]], f16, tag="ascr")
        nc.scalar.activation(
            s[:],
            src,
            mybir.ActivationFunctionType.Square,
            accum_out=acc_col(q),
        )

    def dve_stt(a, b, q):
        # single-instruction multiply+accumulate (1x mode; tiny spans only)
        s = scr["dve"].tile([P, a.shape[-1]], f16, tag="dscr")
        nc.vector.scalar_tensor_tensor(
            out=s[:],
            in0=a,
            scalar=1.0,
            in1=b,
            op0=mybir.AluOpType.mult,
            op1=mybir.AluOpType.mult,
            accum_out=acc_col(q),
        )

    # one PSUM bank per era: [pr | sq | colsums]
    est = [
        psp.tile([P, 128], f32, name=f"est{t}", tag=f"est{t}") for t in range(NT)
    ]
    PRW = (32, 32, 32, 20)  # pr sub-width per era
    SQOF = (32, 32, 32, 20)  # sq offset = pr width
    CSOF = (64, 64, 64, 34)  # colsum offset
    EW = (128, 128, 128, 98)  # used width
    fill = [{"pr": 0, "sq": 0} for _ in range(NT)]

    def pe_reduce(s, t, kind):
        cw = s.shape[-1]
        base = fill[t][kind] + (0 if kind == "pr" else SQOF[t])
        fill[t][kind] += cw // P
        assert fill[t]["pr"] <= PRW[t] and fill[t]["sq"] <= CSOF[t] - SQOF[t]
        for j in range(cw // P):
            nc.tensor.matmul(
                est[t][:, base + j : base + j + 1],
                s[:, j * P : (j + 1) * P],
                ones[:],
            )

    def form(engine, a, b, t, kind):
        pool = scr["dve"] if engine == "dve" else scr["pool"]
        s = pool.tile([P, a.shape[-1]], f16, tag=engine + "scr")
        if engine == "dve":
            nc.vector.tensor_tensor(out=s[:], in0=a, in1=b, op=mybir.AluOpType.mult)
        else:
            nc.gpsimd.tensor_tensor(out=s[:], in0=a, in1=b, op=mybir.AluOpType.mult)
        pe_reduce(s, t, kind)

    def colsums(t, jlo, jhi):
        p1t, p2t = tiles[t]
        base = CSOF[t]
        for j in range(jlo, jhi):
            nc.tensor.matmul(
                est[t][:, base + j : base + j + 1],
                p1t[:, j * P : (j + 1) * P],
                ones[:],
            )
            nc.tensor.matmul(
                est[t][:, base + NCH + j : base + NCH + j + 1],
                p2t[:, j * P : (j + 1) * P],
                ones[:],
            )

    def copy_era(t):
        sb = E3 if t == 3 else EB[t]
        nc.vector.tensor_copy(out_sb[:, sb : sb + EW[t]], est[t][:, 0 : EW[t]])

    tiles = []
    for t in range(NT):
        p1t = pool1.tile([P, D], f16, tag="p1t")
        p2t = pool2.tile([P, D], f16, tag="p2t")
        tiles.append((p1t, p2t))

    def dma_span(t, sl):
        rows = slice(t * P, (t + 1) * P)
        nc.sync.dma_start(out=tiles[t][0][:, sl], in_=p1[rows, sl])
        nc.sync.dma_start(out=tiles[t][1][:, sl], in_=p2[rows, sl])

    H1, H2 = slice(0, 2048), slice(2048, 4096)

    # --- tile 0 DMAs first so transfers start before any setup ops ---
    dma_span(0, H1)
    dma_span(0, H2)
    nc.vector.memset(ones[:], 1.0)
    # unused direct-accum columns must read as zero in the host combine
    nc.gpsimd.memset(out_sb[:, 0:E3], 0.0)
    # preload the ACT table during the DMA head (dummy square on ones)
    warm = misc.tile([P, 1], f16)
    nc.scalar.activation(warm[:], ones[:], mybir.ActivationFunctionType.Square)

    p1t, p2t = tiles[0]
    act_square(p1t[:, H1], "sq1")
    form("dve", p1t[:, H1], p2t[:, H1], 0, "pr")
    form("pool", p2t[:, H1], p2t[:, H1], 0, "sq")
    colsums(0, 0, 16)
    act_square(p1t[:, H2], "sq1")
    form("dve", p1t[:, H2], p2t[:, H2], 0, "pr")
    form("pool", p2t[:, H2], p2t[:, H2], 0, "sq")
    colsums(0, 16, 32)

    # --- tile 1 ---
    dma_span(1, H1)
    dma_span(1, H2)
    p1t, p2t = tiles[1]
    act_square(p1t[:, H1], "sq1")
    form("dve", p1t[:, H1], p2t[:, H1], 1, "pr")
    form("pool", p2t[:, H1], p2t[:, H1], 1, "sq")
    colsums(1, 0, 16)
    act_square(p1t[:, H2], "sq1")
    form("dve", p1t[:, H2], p2t[:, H2], 1, "pr")
    form("pool", p2t[:, H2], p2t[:, H2], 1, "sq")
    colsums(1, 16, 32)

    # --- tile 2 ---
    dma_span(2, H1)
    dma_span(2, H2)
    p1t, p2t = tiles[2]
    act_square(p1t[:, H1], "sq1")
    form("dve", p1t[:, H1], p2t[:, H1], 2, "pr")
    form("dve", p2t[:, H1], p2t[:, H1], 2, "sq")
    copy_era(0)  # POOL tile-0 forms are reduced by now
    colsums(2, 0, 16)
    act_square(p1t[:, H2], "sq1")
    form("dve", p1t[:, H2], p2t[:, H2], 2, "pr")
    form("dve", p2t[:, H2], p2t[:, H2], 2, "sq")
    colsums(2, 16, 32)

    # --- tile 3 (tail): tapered spans, quantities balanced across A/D ---
    t = 3
    p1t, p2t = tiles[t]
    widths = (1024, 768, 768, 512, 384, 384, 256)
    sls = []
    off = 0
    for cw in widths:
        sl = slice(off, off + cw)
        off += cw
        sls.append(sl)
        dma_span(t, sl)
    s0, s1, s2, s3, s4, s5, s6 = sls

    # s0 (1024): chunks 0:8
    act_square(p1t[:, s0], "sq1")
    form("dve", p1t[:, s0], p2t[:, s0], 3, "pr")
    form("dve", p2t[:, s0], p2t[:, s0], 3, "sq")
    colsums(3, 0, 8)
    # s1 (768): chunks 8:14
    form("dve", p1t[:, s1], p1t[:, s1], 3, "sq")  # p1^2 via DVE self-mult
    form("dve", p1t[:, s1], p2t[:, s1], 3, "pr")
    act_square(p2t[:, s1], "sq2")
    colsums(3, 8, 14)
    copy_era(1)  # POOL tile-1 forms are reduced by now
    # s2 (768): chunks 14:20
    act_square(p1t[:, s2], "sq1")
    form("dve", p1t[:, s2], p2t[:, s2], 3, "pr")
    act_square(p2t[:, s2], "sq2")
    colsums(3, 14, 20)
    copy_era(2)
    # bulk output: every bulk column is written by the copies above
    nc.sync.dma_start(out=out[:, EB[0] : OUT_COLS], in_=out_sb[:, EB[0] : OUT_COLS])
    # s3 (512): chunks 20:24
    act_square(p1t[:, s3], "sq1")
    dve_stt(p1t[:, s3], p2t[:, s3], "prd")
    dve_stt(p2t[:, s3], p2t[:, s3], "sq2")
    colsums(3, 20, 24)
    # s4 (384): chunks 24:27
    act_square(p2t[:, s4], "sq2")
    dve_stt(p1t[:, s4], p2t[:, s4], "prd")
    dve_stt(p1t[:, s4], p1t[:, s4], "sq1")
    colsums(3, 24, 27)
    # s5 (384): chunks 27:30
    act_square(p1t[:, s5], "sq1")
    dve_stt(p1t[:, s5], p2t[:, s5], "prd")
    dve_stt(p2t[:, s5], p2t[:, s5], "sq2")
    colsums(3, 27, 30)
    # s6 (256, last bytes): chunks 30:32
    act_square(p1t[:, s6], "sq1")
    dve_stt(p1t[:, s6], p2t[:, s6], "prd")
    dve_stt(p2t[:, s6], p2t[:, s6], "sq2")
    colsums(3, 30, 32)

    # era-3 bank + tiny late DMA for the tail-written columns
    copy_era(3)
    nc.sync.dma_start(out=out[:, 0:TAIL_END], in_=out_sb[:, 0:TAIL_END])


def _get_program():
    if "nc" not in _CACHE:
        _CACHE["nc"] = build_program()
    return _CACHE["nc"]


def run_device(p1, p2, trace=False):
    """Run the SPMD kernel; returns (per-core outs list, BassKernelResults)."""
    nc = _get_program()
    h1 = p1.astype(np.float16)
    h2 = p2.astype(np.float16)
    in_maps = [
        {
            "p1": np.ascontiguousarray(h1[c * RB : (c + 1) * RB]),
            "p2": np.ascontiguousarray(h2[c * RB : (c + 1) * RB]),
        }
        for c in range(N_CORES)
    ]
    try:
        bres = run_bass_kernel_spmd(nc, in_maps, list(range(N_CORES)), trace=trace)
    except ModuleNotFoundError:
        # axon NTFF profile hook unavailable in this image; run untraced
        import os

        os.environ["BASS_NEVER_TRACE"] = "1"
        bres = run_bass_kernel_spmd(nc, in_maps, list(range(N_CORES)), trace=False)
    except Exception:
        # transient device wedge (NRT_EXEC_UNIT_UNRECOVERABLE) recovers after
        # a short wait; retry once before giving up
        import time

        time.sleep(30)
        bres = run_bass_kernel_spmd(nc, in_maps, list(range(N_CORES)), trace=False)
    return [r["out"] for r in bres.results], bres


def combine_partials(outs):
    """float64 combine of the per-core [P, OUT_COLS] partials -> f32 scalar."""
    total = np.zeros((P, OUT_COLS), np.float64)
    for o in outs:
        total += o.astype(np.float64)

    n1 = total[:, DIR_SQ1:DIR_SQ2].sum() + total[:, E3 + 28 : E3 + 34].sum()
    n2 = (
        total[:, DIR_SQ2:DIR_PRD].sum()
        + total[:, E3 + 20 : E3 + 28].sum()
        + sum(total[:, b + 32 : b + 64].sum() for b in EB)
    )
    pp = (
        total[:, DIR_PRD : DIR_PRD + 6].sum()
        + total[:, E3 : E3 + 20].sum()
        + sum(total[:, b : b + 32].sum() for b in EB)
    )

    cs = total[:, E3 + 34 : E3 + 98]
    for b in EB:
        cs = cs + total[:, b + 64 : b + 128]
    s1 = cs[:, 0:NCH].T.reshape(-1)  # colsum(p1), index j*128+m
    s2 = cs[:, NCH : 2 * NCH].T.reshape(-1)

    S = n1 + n2 - 2.0 * pp  # sum((p1-p2)^2) == trace(d)
    d_sum = B * (n1 + n2) - 2.0 * (s1 @ s2)
    off = d_sum - S
    result = S / B - off / (B * (B - 1))
    return np.asarray(result, dtype=np.float32)


def kernel(postive1, postive2):
    p1 = np.ascontiguousarray(np.asarray(postive1, dtype=np.float32))
    p2 = np.ascontiguousarray(np.asarray(postive2, dtype=np.float32))
    assert p1.shape == (B, D) and p2.shape == (B, D)
    outs, _ = run_device(p1, p2, trace=False)
    return combine_partials(outs)


# revision 19
# speedup vs baseline: 1.1059x; 1.0557x over previous
"""ContrastiveLoss (nn_ContrastiveLoss_17093969838495) Trainium2 kernel.

Math: for p1, p2 in R^{BxD} the reference computes
    pos_loss = sum((p1-p2)^2)/B
    d[i,j]   = ||p1_i||^2 + ||p2_j||^2 - 2 <p1_i, p2_j>
    neg_loss = -(sum(d) - trace(d)) / (B*(B-1))
    out      = pos_loss + neg_loss

The BxB matrix is never needed:
    sum(d)   = B*sum(p1^2) + B*sum(p2^2) - 2 * (colsum(p1) . colsum(p2))
    trace(d) = sum(p1^2) + sum(p2^2) - 2*sum(p1 * p2) = sum((p1-p2)^2)

So each core only reduces its 512-row block: sums of squares, sum of
products, and per-column sums. The kernel is input-DMA bound, so inputs
are cast to fp16 on the host before transfer (quantizing inputs AND
products to fp16 moves the final loss by ~3e-6 relative; the gate is
2e-2). All on-device accumulation stays fp32.

Engine split (fp16 keeps DVE in its fast 2x tensor_tensor mode;
scalar_tensor_tensor is 1x-only, so it is used only on tiny tail spans):
  ACT : forms p1^2 via Square (no accumulate — the 187ns accumulator
        read per op is saved by letting PE reduce the formed tile), and
        copies each era's PSUM bank to the output tile (ACT reads PSUM;
        putting copies on DVE lets the tile scheduler block DVE's
        compute stream behind POOL-dependent waits)
  DVE : forms p1*p2 and leftover squares with 2x TT; STT on tail spans
  POOL: forms p2^2 for the first two row-tiles (otherwise idle)
  PE  : reduces every formed tile AND the raw colsums with
        data-stationary ones-vector matmuls into separate PSUM columns
        (PSUM matmul accumulation flags corrupt data on this toolchain,
        so every matmul gets its own column and the HOST does the final
        fold).
PSUM is bank-granular: each era (row-tile) owns one PSUM bank laid out
[products | p2^2 | p1^2 | colsums]. All compute is per-span so each op
starts as soon as its bytes land; the final tile's spans taper so the
tail past the last byte stays short. Tail-written output columns sit
contiguously in [0:158) and leave in a tiny late DMA; the bulk leaves
early. Host combines the 8 per-core [128, OUT_COLS] partials in f64.
"""

import numpy as np

try:
    import concourse.bass as bass
except ImportError:  # pragma: no cover - path fallback for fresh dirs
    import sys

    sys.path.insert(0, "/opt/trn_rl_repo")
    import concourse.bass as bass

import concourse.bacc as bacc
import concourse.tile as tile
from concourse import mybir
from concourse.bass_utils import run_bass_kernel_spmd

N_CORES = 8
B = 4096
D = 4096
RB = B // N_CORES  # 512 rows per core
P = 128  # SBUF partitions
NT = RB // P  # 4 row-tiles per core
NCH = D // P  # 32 column chunks of 128

# ---- PSUM era-bank layouts: [pr | sq2 | sq1 | cs] ----
SQ2OF = (32, 32, 32, 20)
SQ1OF = (64, 64, 64, 47)
CSOF = (96, 96, 96, 70)
EW = (160, 160, 160, 134)

# ---- output layout (tail-written columns first, bulk after) ----
DIR_SQ1, DIR_SQ2, DIR_PRD = 0, 8, 16  # direct accum columns, 8 each
E3 = 24  # era-3 bank image [24:158)
TAIL_END = 24 + EW[3]  # 158
EB = (160, 320, 480)  # era 0-2 bank images
OUT_COLS = 640

_CACHE = {}


def build_program(replicas=1):
    f16 = mybir.dt.float16
    f32 = mybir.dt.float32
    nc = bacc.Bacc(
        "TRN2", target_bir_lowering=False, debug=False, num_devices=N_CORES
    )
    p1 = nc.dram_tensor("p1", [RB, D], f16, kind="ExternalInput")
    p2 = nc.dram_tensor("p2", [RB, D], f16, kind="ExternalInput")
    out = nc.dram_tensor("out", [P, OUT_COLS], f32, kind="ExternalOutput")

    with tile.TileContext(nc) as tc:
        with (
            tc.tile_pool(name="in1", bufs=4) as pool1,
            tc.tile_pool(name="in2", bufs=4) as pool2,
            tc.tile_pool(name="ascr", bufs=2) as ascr,
            tc.tile_pool(name="dscr", bufs=3) as dscr,
            tc.tile_pool(name="pscr", bufs=3) as pscr,
            tc.tile_pool(name="misc", bufs=1) as misc,
            tc.tile_pool(name="outp", bufs=2) as outp,
            tc.tile_pool(name="psum", bufs=1, space=bass.MemorySpace.PSUM) as psp,
        ):
            scr = {"act": ascr, "dve": dscr, "pool": pscr}
            ones = misc.tile([P, 1], f16)
            for _rep in range(replicas):
                _build_body(nc, pool1, pool2, scr, misc, outp, psp, ones, p1, p2, out)

    nc.compile()
    return nc


def _build_body(nc, pool1, pool2, scr, misc, outp, psp, ones, p1, p2, out):
    f16 = mybir.dt.float16
    f32 = mybir.dt.float32
    out_sb = outp.tile([P, OUT_COLS], f32, tag="out_sb")

    dir_n = {"sq1": DIR_SQ1, "sq2": DIR_SQ2, "prd": DIR_PRD}

    def acc_col(q):
        c = dir_n[q]
        dir_n[q] += 1
        assert c < (8, 16, 24)[("sq1", "sq2", "prd").index(q)], q
        return out_sb[:, c : c + 1]

    def act_square_acc(src, q):
        s = scr["act"].tile([P, src.shape[-1]], f16, tag="ascr")
        nc.scalar.activation(
            s[:],
            src,
            mybir.ActivationFunctionType.Square,
            accum_out=acc_col(q),
        )

    def dve_stt(a, b, q):
        # single-instruction multiply+accumulate (1x mode; tiny spans only)
        s = scr["dve"].tile([P, a.shape[-1]], f16, tag="dscr")
        nc.vector.scalar_tensor_tensor(
            out=s[:],
            in0=a,
            scalar=1.0,
            in1=b,
            op0=mybir.AluOpType.mult,
            op1=mybir.AluOpType.mult,
            accum_out=acc_col(q),
        )

    # one PSUM bank per era: [pr | sq2 | sq1 | cs]
    est = [
        psp.tile([P, 224], f32, name=f"est{t}", tag=f"est{t}") for t in range(NT)
    ]
    fill = [{"pr": 0, "sq2": 0, "sq1": 0} for _ in range(NT)]

    def pe_reduce(s, t, kind):
        off = {"pr": 0, "sq2": SQ2OF[t], "sq1": SQ1OF[t]}[kind]
        lim = {"pr": SQ2OF[t], "sq2": SQ1OF[t] - SQ2OF[t], "sq1": CSOF[t] - SQ1OF[t]}
        cw = s.shape[-1]
        base = off + fill[t][kind]
        fill[t][kind] += cw // P
        assert fill[t][kind] <= lim[kind], (t, kind)
        for j in range(cw // P):
            nc.tensor.matmul(
                est[t][:, base + j : base + j + 1],
                s[:, j * P : (j + 1) * P],
                ones[:],
            )

    def act_square(src, t):
        # form p1^2 on ACT (no accumulate), PE reduces it
        s = scr["act"].tile([P, src.shape[-1]], f16, tag="ascr")
        nc.scalar.activation(s[:], src, mybir.ActivationFunctionType.Square)
        pe_reduce(s, t, "sq1")

    def act_square2(src, t):
        # form p2^2 on ACT, PE reduces it (tail spans)
        s = scr["act"].tile([P, src.shape[-1]], f16, tag="ascr")
        nc.scalar.activation(s[:], src, mybir.ActivationFunctionType.Square)
        pe_reduce(s, t, "sq2")

    def form(engine, a, b, t, kind):
        pool = scr["dve"] if engine == "dve" else scr["pool"]
        s = pool.tile([P, a.shape[-1]], f16, tag=engine + "scr")
        if engine == "dve":
            nc.vector.tensor_tensor(out=s[:], in0=a, in1=b, op=mybir.AluOpType.mult)
        else:
            nc.gpsimd.tensor_tensor(out=s[:], in0=a, in1=b, op=mybir.AluOpType.mult)
        pe_reduce(s, t, kind)

    def colsums(t, jlo, jhi):
        p1t, p2t = tiles[t]
        base = CSOF[t]
        for j in range(jlo, jhi):
            nc.tensor.matmul(
                est[t][:, base + j : base + j + 1],
                p1t[:, j * P : (j + 1) * P],
                ones[:],
            )
            nc.tensor.matmul(
                est[t][:, base + NCH + j : base + NCH + j + 1],
                p2t[:, j * P : (j + 1) * P],
                ones[:],
            )

    def copy_era(t):
        # ACT copies the era bank (keeps POOL-dependent waits off DVE)
        sb = E3 if t == 3 else EB[t]
        nc.scalar.activation(
            out_sb[:, sb : sb + EW[t]],
            est[t][:, 0 : EW[t]],
            mybir.ActivationFunctionType.Copy,
        )

    tiles = []
    for t in range(NT):
        p1t = pool1.tile([P, D], f16, tag="p1t")
        p2t = pool2.tile([P, D], f16, tag="p2t")
        tiles.append((p1t, p2t))

    def dma_span(t, sl):
        rows = slice(t * P, (t + 1) * P)
        nc.sync.dma_start(out=tiles[t][0][:, sl], in_=p1[rows, sl])
        nc.sync.dma_start(out=tiles[t][1][:, sl], in_=p2[rows, sl])

    H1, H2 = slice(0, 2048), slice(2048, 4096)

    # --- tile 0 DMAs first so transfers start before any setup ops ---
    dma_span(0, H1)
    dma_span(0, H2)
    nc.vector.memset(ones[:], 1.0)
    # unused direct-accum columns must read as zero in the host combine
    nc.gpsimd.memset(out_sb[:, 0:E3], 0.0)
    # preload the ACT table during the DMA head (dummy square on ones)
    warm = misc.tile([P, 1], f16)
    nc.scalar.activation(warm[:], ones[:], mybir.ActivationFunctionType.Square)

    p1t, p2t = tiles[0]
    act_square(p1t[:, H1], 0)
    form("dve", p1t[:, H1], p2t[:, H1], 0, "pr")
    form("pool", p2t[:, H1], p2t[:, H1], 0, "sq2")
    colsums(0, 0, 16)
    act_square(p1t[:, H2], 0)
    form("dve", p1t[:, H2], p2t[:, H2], 0, "pr")
    form("pool", p2t[:, H2], p2t[:, H2], 0, "sq2")
    colsums(0, 16, 32)

    # --- tile 1 ---
    dma_span(1, H1)
    dma_span(1, H2)
    p1t, p2t = tiles[1]
    act_square(p1t[:, H1], 1)
    form("dve", p1t[:, H1], p2t[:, H1], 1, "pr")
    form("pool", p2t[:, H1], p2t[:, H1], 1, "sq2")
    colsums(1, 0, 16)
    act_square(p1t[:, H2], 1)
    form("dve", p1t[:, H2], p2t[:, H2], 1, "pr")
    form("pool", p2t[:, H2], p2t[:, H2], 1, "sq2")
    colsums(1, 16, 32)

    # --- tile 2 ---
    dma_span(2, H1)
    dma_span(2, H2)
    p1t, p2t = tiles[2]
    act_square(p1t[:, H1], 2)
    form("dve", p1t[:, H1], p2t[:, H1], 2, "pr")
    form("dve", p2t[:, H1], p2t[:, H1], 2, "sq2")
    copy_era(0)  # POOL tile-0 forms are reduced by now
    colsums(2, 0, 16)
    act_square(p1t[:, H2], 2)
    form("dve", p1t[:, H2], p2t[:, H2], 2, "pr")
    form("dve", p2t[:, H2], p2t[:, H2], 2, "sq2")
    colsums(2, 16, 32)

    # --- tile 3 (tail): tapered spans, quantities balanced across A/D ---
    t = 3
    p1t, p2t = tiles[t]
    widths = (1024, 768, 768, 512, 384, 384, 256)
    sls = []
    off = 0
    for cw in widths:
        sl = slice(off, off + cw)
        off += cw
        sls.append(sl)
        dma_span(t, sl)
    s0, s1, s2, s3, s4, s5, s6 = sls

    # s0 (1024): chunks 0:8
    act_square(p1t[:, s0], 3)
    form("dve", p1t[:, s0], p2t[:, s0], 3, "pr")
    form("dve", p2t[:, s0], p2t[:, s0], 3, "sq2")
    colsums(3, 0, 8)
    # s1 (768): chunks 8:14
    act_square2(p2t[:, s1], 3)
    form("dve", p1t[:, s1], p1t[:, s1], 3, "sq1")
    form("dve", p1t[:, s1], p2t[:, s1], 3, "pr")
    colsums(3, 8, 14)
    copy_era(1)  # POOL tile-1 forms are reduced by now
    # s2 (768): chunks 14:20
    act_square(p1t[:, s2], 3)
    form("dve", p1t[:, s2], p2t[:, s2], 3, "pr")
    form("dve", p2t[:, s2], p2t[:, s2], 3, "sq2")
    colsums(3, 14, 20)
    copy_era(2)
    # bulk output: every bulk column is written by the copies above
    nc.sync.dma_start(out=out[:, EB[0] : OUT_COLS], in_=out_sb[:, EB[0] : OUT_COLS])
    # s3 (512): chunks 20:24
    act_square2(p2t[:, s3], 3)
    dve_stt(p1t[:, s3], p1t[:, s3], "sq1")
    dve_stt(p1t[:, s3], p2t[:, s3], "prd")
    colsums(3, 20, 24)
    # s4 (384): chunks 24:27
    act_square(p1t[:, s4], 3)
    dve_stt(p2t[:, s4], p2t[:, s4], "sq2")
    dve_stt(p1t[:, s4], p2t[:, s4], "prd")
    colsums(3, 24, 27)
    # s5 (384): chunks 27:30
    act_square2(p2t[:, s5], 3)
    dve_stt(p1t[:, s5], p1t[:, s5], "sq1")
    dve_stt(p1t[:, s5], p2t[:, s5], "prd")
    colsums(3, 27, 30)
    # s6 (256, last bytes): direct accums only so copy_era(3) can't wait on it
    act_square_acc(p1t[:, s6], "sq1")
    dve_stt(p2t[:, s6], p2t[:, s6], "sq2")
    dve_stt(p1t[:, s6], p2t[:, s6], "prd")
    colsums(3, 30, 32)

    # era-3 bank + tiny late DMA for the tail-written columns
    copy_era(3)
    nc.sync.dma_start(out=out[:, 0:TAIL_END], in_=out_sb[:, 0:TAIL_END])


def _get_program():
    if "nc" not in _CACHE:
        _CACHE["nc"] = build_program()
    return _CACHE["nc"]


def run_device(p1, p2, trace=False):
    """Run the SPMD kernel; returns (per-core outs list, BassKernelResults)."""
    nc = _get_program()
    h1 = p1.astype(np.float16)
    h2 = p2.astype(np.float16)
    in_maps = [
        {
            "p1": np.ascontiguousarray(h1[c * RB : (c + 1) * RB]),
            "p2": np.ascontiguousarray(h2[c * RB : (c + 1) * RB]),
        }
        for c in range(N_CORES)
    ]
    try:
        bres = run_bass_kernel_spmd(nc, in_maps, list(range(N_CORES)), trace=trace)
    except ModuleNotFoundError:
        # axon NTFF profile hook unavailable in this image; run untraced
        import os

        os.environ["BASS_NEVER_TRACE"] = "1"
        bres = run_bass_kernel_spmd(nc, in_maps, list(range(N_CORES)), trace=False)
    except Exception:
        # transient device wedge (NRT_EXEC_UNIT_UNRECOVERABLE) recovers after
        # a short wait; retry once before giving up
        import time

        time.sleep(30)
        bres = run_bass_kernel_spmd(nc, in_maps, list(range(N_CORES)), trace=False)
    return [r["out"] for r in bres.results], bres


def combine_partials(outs):
    """float64 combine of the per-core [P, OUT_COLS] partials -> f32 scalar."""
    total = np.zeros((P, OUT_COLS), np.float64)
    for o in outs:
        total += o.astype(np.float64)

    def era(t, lo, hi):
        b = E3 if t == 3 else EB[t]
        return total[:, b + lo : b + hi]

    n1 = total[:, DIR_SQ1:DIR_SQ2].sum() + sum(
        era(t, SQ1OF[t], CSOF[t]).sum() for t in range(NT)
    )
    n2 = total[:, DIR_SQ2:DIR_PRD].sum() + sum(
        era(t, SQ2OF[t], SQ1OF[t]).sum() for t in range(NT)
    )
    pp = total[:, DIR_PRD:E3].sum() + sum(
        era(t, 0, SQ2OF[t]).sum() for t in range(NT)
    )

    cs = sum(era(t, CSOF[t], CSOF[t] + 2 * NCH) for t in range(NT))  # [128, 64]
    s1 = cs[:, 0:NCH].T.reshape(-1)  # colsum(p1), index j*128+m
    s2 = cs[:, NCH : 2 * NCH].T.reshape(-1)

    S = n1 + n2 - 2.0 * pp  # sum((p1-p2)^2) == trace(d)
    d_sum = B * (n1 + n2) - 2.0 * (s1 @ s2)
    off = d_sum - S
    result = S / B - off / (B * (B - 1))
    return np.asarray(result, dtype=np.float32)


def kernel(postive1, postive2):
    p1 = np.ascontiguousarray(np.asarray(postive1, dtype=np.float32))
    p2 = np.ascontiguousarray(np.asarray(postive2, dtype=np.float32))
    assert p1.shape == (B, D) and p2.shape == (B, D)
    outs, _ = run_device(p1, p2, trace=False)
    return combine_partials(outs)
